# revision 20
# baseline (speedup 1.0000x reference)
"""Batched 20x20 SPD covariance-matrix inversion on 8 Trainium2 NeuronCores.

For each of 131072 batches: build C = exp(-1.5 * pairwise_dist(pos)) + 0.01*I
from 20 2-D points, return C^{-1}.

Strategy (per core, data-parallel over batch):
 - batch-major layout: each of 128 SBUF partitions holds M matrices' full
   20x20 (400 fp32) in the free dim; 3 chunks (M = 44/42/42) per partition
   (fewer chunks amortize the fixed ~58-cycle DVE per-op init over more
   matrices; 3 is the most that fits two M-sized A tiles in SBUF).
 - symmetric sweep operator (Gauss-Jordan preserving symmetry): only the
   upper triangle is updated each pivot, covered by 2-row rectangles
   (minimizes wasted elements vs per-op overhead on the DVE).
 - the covariance is built on the upper triangle only; the nugget TAU is
   not stored in the matrix but added when the pivot reciprocal is taken
   (exact: the diagonal offset rides additively through the sweep).
 - the final negate is folded into pivot 19 by reversing the subtraction;
   the lower-triangle mirror runs on the ACT engine interleaved with
   pivot 19's updates, and the output DMA is split into row-halves so the
   first half streams out while the second is still being updated.
 - DVE does only the reciprocal, c*r scaling, and the rank-1 updates; all
   gathers (split per-rect so they complete during the previous pivot's
   updates), the nugget add, pivot-slot zeroing, diagonal/pivot-row/col
   writes, and the mirror run on the otherwise-idle ACT engine.
 - table-sensitive ACT ops (Square/Sqrt/Exp) are chained in emission order
   so the scheduler cannot interleave Sqrt and Exp (a table-set switch
   costs ~2.7us); Copy/Identity ops live in every set and schedule freely.
 - each chunk's build is emitted during the previous chunk's final pivot
   so both engines stay busy across chunk transitions.
"""

import numpy as np

import concourse.bass as bass  # noqa: F401  (registers engine APIs)
import concourse.tile as tile
from concourse import bacc, mybir
from concourse.bass_utils import run_bass_kernel_spmd

N = 20                  # matrix dim
D = 2                   # coord dim
PHI = 1.5
TAU = 0.01
P = 128                 # SBUF partitions
N_CORES = 8
B_TOTAL = 131072
B_CORE = B_TOTAL // N_CORES   # 16384

F32 = mybir.dt.float32
AF = mybir.ActivationFunctionType
OP = mybir.AluOpType

RECT_H = 2              # rect-cover row height (2 minimizes DVE cycles)


def emit_kernel(tc, pos_ap, out_ap, b_core, m_list):
    """Emit the per-core program. pos: [b_core, 40] f32, out: [b_core, 400]
    f32.  m_list: matrices per partition per chunk (sum = b_core // 128)."""
    nc = tc.nc
    T = b_core // P
    assert sum(m_list) == T and b_core == P * T
    MX = max(m_list)
    chunks = len(m_list)
    offs = [sum(m_list[:i]) for i in range(chunks)]
    rects = [(a, min(a + RECT_H, N)) for a in range(0, N, RECT_H)]

    def act(fn, *args, **kw):
        """ACT op whose function (Copy/Identity) is present in every
        activation table set — safe to schedule freely."""
        return fn(*args, **kw)

    def actT(fn, *args, **kw):
        """Table-sensitive ACT op (Square/Sqrt/Exp), chained in emission
        order so the scheduler cannot interleave Sqrt and Exp (each
        table-set switch costs ~2.7us)."""
        inst = fn(*args, **kw)
        tc.chain_iter_dep("actq_t", inst.ins)
        return inst

    pos_v = pos_ap.rearrange("(p t) f -> p (t f)", p=P)
    out_v = out_ap.rearrange("(p t) (i j) -> p t i j", p=P, i=N)

    with (
        tc.tile_pool(name="const", bufs=1) as const_pool,
        tc.tile_pool(name="pos", bufs=2) as pos_pool,
        tc.tile_pool(name="A", bufs=2) as a_pool,
        tc.tile_pool(name="dy", bufs=2) as dy_pool,
        tc.tile_pool(name="rect", bufs=2) as rect_pool,
        tc.tile_pool(name="small", bufs=2) as small_pool,
    ):
        tau_t = const_pool.tile([P, 1], F32)
        nc.vector.memset(tau_t[:, :], TAU)

        states = {}

        def alloc_piv(st, k):
            M = st["M"]
            cK = small_pool.tile([P, MX * N], F32, tag="c")
            crK = small_pool.tile([P, MX * N], F32, tag="cr")
            rK = small_pool.tile([P, MX], F32, tag="r")
            rT = small_pool.tile([P, MX], F32, tag="rt")
            st["piv"][k] = (
                cK[:, : M * N].rearrange("p (m i) -> p m i", m=M),
                crK[:, : M * N].rearrange("p (m i) -> p m i", m=M),
                rK[:, :M], rT[:, :M],
            )

        def prep_piv(st, k):
            """Row-part gather + nugget add + pivot-slot zero for pivot k
            (ACT).  Emitted as soon as row k's update has been emitted so
            the DVE reciprocal never waits."""
            A4 = st["A4"]
            c3, _, _, rT = st["piv"][k]
            act(nc.scalar.copy, c3[:, :, k:], A4[:, :, k, k:])
            act(nc.scalar.activation, rT, c3[:, :, k], AF.Identity, tau_t[:, :])
            act(nc.scalar.mul, c3[:, :, k], c3[:, :, k], 0.0)  # zero slot

        def build(c):
            """Covariance build for chunk c (upper rects only):
            A = d^2 -> d -> exp, plus pivot-0 gather/prep.  Called one
            pivot early (during chunk c-1's last pivot) so the chunk
            transition keeps both engines busy."""
            M, off = m_list[c], offs[c]
            pos_t = pos_pool.tile([P, MX * N * D], F32)
            nc.sync.dma_start(
                pos_t[:, : M * N * D], pos_v[:, off * N * D : (off + M) * N * D]
            )
            posv = pos_t[:, : M * N * D].rearrange("p (m i d) -> p m i d", m=M, i=N)
            A = a_pool.tile([P, MX * N * N], F32)
            A4 = A[:, : M * N * N].rearrange("p (m i j) -> p m i j", m=M, i=N)
            st = {"A4": A4, "piv": {}, "M": M, "off": off}
            states[c] = st

            for idx, (r0, r1) in enumerate(rects):
                nr, ncl = r1 - r0, N - r0
                reg = A4[:, :, r0:r1, r0:]
                xi = posv[:, :, r0:r1, 0].unsqueeze(3).broadcast_to([P, M, nr, ncl])
                xj = posv[:, :, r0:, 0].unsqueeze(2).broadcast_to([P, M, nr, ncl])
                nc.vector.tensor_sub(reg, xi, xj)
                dy = dy_pool.tile([P, MX * RECT_H * N], F32, tag="dy")
                dyv = dy[:, : M * nr * ncl].rearrange(
                    "p (m i j) -> p m i j", m=M, i=nr
                )
                yi = posv[:, :, r0:r1, 1].unsqueeze(3).broadcast_to([P, M, nr, ncl])
                yj = posv[:, :, r0:, 1].unsqueeze(2).broadcast_to([P, M, nr, ncl])
                nc.vector.tensor_sub(dyv, yi, yj)
                actT(nc.scalar.square, reg, reg)
                actT(nc.scalar.square, dyv, dyv)
                nc.vector.tensor_add(reg, reg, dyv)
                # sqrt interleaved per rect: Square and Sqrt share table
                # sets, so the interleave costs no extra table loads.
                actT(nc.scalar.sqrt, reg, reg)
                if idx == 0:
                    # exp + pivot-0 gather/prep for rect 0 immediately, so
                    # pivot 0's reciprocal input is ready long before the
                    # build's DVE work finishes (costs 2 extra table-set
                    # loads per chunk on the idle ACT engine).
                    actT(nc.scalar.activation, reg, reg, AF.Exp, scale=-PHI)
                    alloc_piv(st, 0)
                    prep_piv(st, 0)

        build(0)
        for c in range(chunks):
            st = states.pop(c)
            A4 = st["A4"]
            piv = st["piv"]
            M, off = st["M"], st["off"]

            def rect_reg(r0, r1):
                return A4[:, :, r0:r1, r0:]

            # ---- sweep all 20 pivots ----
            for k in range(N):
                last = k == N - 1
                if last and c + 1 < chunks:
                    # emit the next chunk's build here: its DVE work fills
                    # the gap while ACT runs this chunk's tail + its own
                    # sqrt/exp, and pivot 0 of the next chunk starts with
                    # everything prepped.
                    build(c + 1)
                c3, cr3, rK, rT = piv.pop(k)

                # r = 1/(A[k,k] + TAU)  (nugget applied at pivot time)
                nc.vector.reciprocal(rK, rT)
                # diag <- -r (the final pivot's reversed update negates it)
                act(nc.scalar.mul, A4[:, :, k, k], rK, -1.0)

                if k == 0:
                    # reversed: smallest rects' exps first, matching pivot
                    # 0's rect execution order so ACT stays ahead of DVE
                    for (r0_, r1_) in rects[:0:-1]:
                        actT(nc.scalar.activation,
                            rect_reg(r0_, r1_), rect_reg(r0_, r1_), AF.Exp,
                            scale=-PHI)

                rb = rK.unsqueeze(2).broadcast_to([P, M, N])
                nc.vector.tensor_mul(cr3, c3, rb)

                if last:
                    # pivot col <- cr before the reversed update negates it
                    act(nc.scalar.copy, A4[:, :, :k, k], cr3[:, :, :k])

                # rank-1 update of the upper triangle (rect cover); at the
                # final pivot the subtraction is reversed, which emits the
                # negated matrix (= the inverse) directly.  The rect holding
                # row k runs first (then the one holding row k+1) so the
                # pivot-row write and the next pivot's gather/prep (ACT)
                # complete while the remaining rects are still running.
                first = list(dict.fromkeys(
                    [k // 2, min((k + 1) // 2, len(rects) - 1)]
                ))
                if k == 0:
                    # descending sizes after rect 0: the exp for each rect
                    # (produced smallest-first on ACT) is ready in time
                    order = first + list(range(len(rects) - 1, 0, -1))
                else:
                    order = first + [
                        j for j in range(len(rects)) if j not in first
                    ]
                for ri in order:
                    r0, r1 = rects[ri]
                    nr, ncl = r1 - r0, N - r0
                    tmp = rect_pool.tile([P, MX * RECT_H * N], F32, tag="rect")
                    tv = tmp[:, : M * nr * ncl].rearrange(
                        "p (m i j) -> p m i j", m=M, i=nr
                    )
                    cb = c3[:, :, r0:r1].unsqueeze(3).broadcast_to([P, M, nr, ncl])
                    crb = cr3[:, :, r0:].unsqueeze(2).broadcast_to([P, M, nr, ncl])
                    reg = rect_reg(r0, r1)
                    nc.vector.tensor_mul(tv, cb, crb)
                    if last:
                        nc.vector.tensor_sub(reg, tv, reg)
                        # mirror the rows this rect finalized (ACT)
                        for i in (2 * ri, 2 * ri + 1):
                            if i < N - 1:
                                act(nc.scalar.copy,
                                    A4[:, :, i + 1 :, i], A4[:, :, i, i + 1 :])
                    else:
                        nc.vector.tensor_sub(reg, reg, tv)
                        if ri == k // 2:
                            # pivot row <- cr.  The rank-1 update left it
                            # unchanged (c[k] = cr[k] = 0), so writing after
                            # the rect is equivalent and runs on ACT.
                            act(nc.scalar.copy,
                                A4[:, :, k, k + 1 :], cr3[:, :, k + 1 :])
                        if ri == (k + 1) // 2:
                            alloc_piv(st, k + 1)
                            prep_piv(st, k + 1)

                if not last:
                    # column-part gather for pivot k+1, split along the rect
                    # cover: each piece depends on a single rect update above
                    # and completes while the later rects are still running.
                    cn3 = piv[k + 1][0]
                    for (r0, r1) in rects:
                        lo, hi = r0, min(r1, k + 1)
                        if lo >= hi:
                            break
                        act(nc.scalar.copy, cn3[:, :, lo:hi],
                            A4[:, :, lo:hi, k + 1])
                    if k:
                        # pivot col <- cr (ACT, after all rects that touch it)
                        act(nc.scalar.copy, A4[:, :, :k, k], cr3[:, :, :k])

            # output DMA split by row-blocks: early rows (updates + mirrors)
            # finalize partway through pivot 19, so their transfer overlaps
            # the remaining updates (matters for the last chunk's exposed
            # tail).
            for (a, b) in ((0, 6), (6, 10), (10, 14), (14, N)):
                nc.sync.dma_start(
                    out_v[:, off : off + M, a:b, :], A4[:, :, a:b, :]
                )


_CACHE = {}


def _m_list_for(b_core, m_chunk):
    T = b_core // P
    if m_chunk is None:
        # 3 chunks: the fewest whose two live A tiles (2 * MX * 1600B) still
        # fit in SBUF alongside the working tiles
        base = [T // 3] * 3
        base[0] += T - 3 * (T // 3)
        return [m for m in base if m]
    return [m_chunk] * (T // m_chunk)


def build_nc(b_core=B_CORE, m_chunk=None, num_devices=N_CORES):
    key = (b_core, m_chunk, num_devices)
    if key in _CACHE:
        return _CACHE[key]
    nc = bacc.Bacc(
        "TRN2", target_bir_lowering=False, debug=False, num_devices=num_devices
    )
    pos_d = nc.dram_tensor("pos", [b_core, N * D], F32, kind="ExternalInput")
    out_d = nc.dram_tensor("out", [b_core, N * N], F32, kind="ExternalOutput")
    with tile.TileContext(nc) as tc:
        emit_kernel(tc, pos_d.ap(), out_d.ap(), b_core, _m_list_for(b_core, m_chunk))
    nc.compile()
    _CACHE[key] = nc
    return nc


def run(pos_full, b_core=B_CORE, m_chunk=None, n_cores=N_CORES, **kw):
    """pos_full: [n_cores*b_core, 20, 2] f32 -> [n_cores*b_core, 20, 20] f32."""
    nc = build_nc(b_core, m_chunk, n_cores)
    flat = np.ascontiguousarray(
        np.asarray(pos_full, dtype=np.float32).reshape(-1, N * D)
    )
    in_maps = [
        {"pos": flat[i * b_core : (i + 1) * b_core]} for i in range(n_cores)
    ]
    res = run_bass_kernel_spmd(nc, in_maps, core_ids=list(range(n_cores)), **kw)
    out = np.concatenate([r["out"] for r in res.results], axis=0)
    return out.reshape(-1, N, N), res


def kernel(neighbor_positions, edge_list=None):
    out, _ = run(neighbor_positions)
    return out


# revision 21
# speedup vs baseline: 1.0131x; 1.0131x over previous
"""Batched 20x20 SPD covariance-matrix inversion on 8 Trainium2 NeuronCores.

For each of 131072 batches: build C = exp(-1.5 * pairwise_dist(pos)) + 0.01*I
from 20 2-D points, return C^{-1}.

Strategy (per core, data-parallel over batch):
 - batch-major layout: each of 128 SBUF partitions holds M matrices' full
   20x20 (400 fp32) in the free dim; 3 chunks (M = 44/42/42) per partition
   (fewer chunks amortize the fixed ~58-cycle DVE per-op init over more
   matrices; 3 is the most that fits two M-sized A tiles in SBUF).
 - symmetric sweep operator (Gauss-Jordan preserving symmetry): only the
   upper triangle is updated each pivot, covered by 2-row rectangles
   (minimizes wasted elements vs per-op overhead on the DVE).
 - the covariance is built on the upper triangle only; the nugget TAU is
   not stored in the matrix but added when the pivot reciprocal is taken
   (exact: the diagonal offset rides additively through the sweep).
 - the final negate is folded into pivot 19 by reversing the subtraction;
   the lower-triangle mirror runs on the ACT engine interleaved with
   pivot 19's updates, and the output DMA is split into row-halves so the
   first half streams out while the second is still being updated.
 - DVE does only the reciprocal, c*r scaling, and the rank-1 updates; all
   gathers (split per-rect so they complete during the previous pivot's
   updates), the nugget add, pivot-slot zeroing, diagonal/pivot-row/col
   writes, and the mirror run on the otherwise-idle ACT engine.
 - table-sensitive ACT ops (Square/Sqrt/Exp) are chained in emission order
   so the scheduler cannot interleave Sqrt and Exp (a table-set switch
   costs ~2.7us); Copy/Identity ops live in every set and schedule freely.
 - each chunk's build is emitted during the previous chunk's final pivot
   so both engines stay busy across chunk transitions.
"""

import numpy as np

import concourse.bass as bass  # noqa: F401  (registers engine APIs)
import concourse.tile as tile
from concourse import bacc, mybir
from concourse.bass_utils import run_bass_kernel_spmd

N = 20                  # matrix dim
D = 2                   # coord dim
PHI = 1.5
TAU = 0.01
P = 128                 # SBUF partitions
N_CORES = 8
B_TOTAL = 131072
B_CORE = B_TOTAL // N_CORES   # 16384

F32 = mybir.dt.float32
AF = mybir.ActivationFunctionType
OP = mybir.AluOpType

RECT_H = 2              # rect-cover row height (2 minimizes DVE cycles)


def emit_kernel(tc, pos_ap, out_ap, b_core, m_list):
    """Emit the per-core program. pos: [b_core, 40] f32, out: [b_core, 400]
    f32.  m_list: matrices per partition per chunk (sum = b_core // 128)."""
    nc = tc.nc
    T = b_core // P
    assert sum(m_list) == T and b_core == P * T
    MX = max(m_list)
    chunks = len(m_list)
    offs = [sum(m_list[:i]) for i in range(chunks)]
    rects = [(a, min(a + RECT_H, N)) for a in range(0, N, RECT_H)]

    def act(fn, *args, **kw):
        """ACT op whose function (Copy/Identity) is present in every
        activation table set — safe to schedule freely."""
        return fn(*args, **kw)

    def actT(fn, *args, **kw):
        """Table-sensitive ACT op (Square/Sqrt/Exp), chained in emission
        order so the scheduler cannot interleave Sqrt and Exp (each
        table-set switch costs ~2.7us)."""
        inst = fn(*args, **kw)
        tc.chain_iter_dep("actq_t", inst.ins)
        return inst

    pos_v = pos_ap.rearrange("(p t) f -> p (t f)", p=P)
    out_v = out_ap.rearrange("(p t) (i j) -> p t i j", p=P, i=N)

    with (
        tc.tile_pool(name="const", bufs=1) as const_pool,
        tc.tile_pool(name="pos", bufs=2) as pos_pool,
        tc.tile_pool(name="A", bufs=2) as a_pool,
        tc.tile_pool(name="dy", bufs=2) as dy_pool,
        tc.tile_pool(name="rect", bufs=2) as rect_pool,
        tc.tile_pool(name="small", bufs=2) as small_pool,
    ):
        tau_t = const_pool.tile([P, 1], F32)
        nc.vector.memset(tau_t[:, :], TAU)

        states = {}

        def alloc_piv(st, k):
            M = st["M"]
            cK = small_pool.tile([P, MX * N], F32, tag="c")
            crK = small_pool.tile([P, MX * N], F32, tag="cr")
            rK = small_pool.tile([P, MX], F32, tag="r")
            rT = small_pool.tile([P, MX], F32, tag="rt")
            st["piv"][k] = (
                cK[:, : M * N].rearrange("p (m i) -> p m i", m=M),
                crK[:, : M * N].rearrange("p (m i) -> p m i", m=M),
                rK[:, :M], rT[:, :M],
            )

        def prep_piv(st, k):
            """Row-part gather + nugget add + pivot-slot zero for pivot k
            (ACT).  Emitted as soon as row k's update has been emitted so
            the DVE reciprocal never waits."""
            A4 = st["A4"]
            c3, _, _, rT = st["piv"][k]
            act(nc.scalar.copy, c3[:, :, k:], A4[:, :, k, k:])
            act(nc.scalar.activation, rT, c3[:, :, k], AF.Identity, tau_t[:, :])
            act(nc.scalar.mul, c3[:, :, k], c3[:, :, k], 0.0)  # zero slot

        def build(c):
            """Covariance build for chunk c (upper rects only):
            A = d^2 -> d -> exp, plus pivot-0 gather/prep.  Called one
            pivot early (during chunk c-1's last pivot) so the chunk
            transition keeps both engines busy."""
            M, off = m_list[c], offs[c]
            pos_t = pos_pool.tile([P, MX * N * D], F32)
            nc.sync.dma_start(
                pos_t[:, : M * N * D], pos_v[:, off * N * D : (off + M) * N * D]
            )
            posv = pos_t[:, : M * N * D].rearrange("p (m i d) -> p m i d", m=M, i=N)
            A = a_pool.tile([P, MX * N * N], F32)
            A4 = A[:, : M * N * N].rearrange("p (m i j) -> p m i j", m=M, i=N)
            st = {"A4": A4, "piv": {}, "M": M, "off": off}
            states[c] = st

            for idx, (r0, r1) in enumerate(rects):
                nr, ncl = r1 - r0, N - r0
                reg = A4[:, :, r0:r1, r0:]
                xi = posv[:, :, r0:r1, 0].unsqueeze(3).broadcast_to([P, M, nr, ncl])
                xj = posv[:, :, r0:, 0].unsqueeze(2).broadcast_to([P, M, nr, ncl])
                nc.vector.tensor_sub(reg, xi, xj)
                dy = dy_pool.tile([P, MX * RECT_H * N], F32, tag="dy")
                dyv = dy[:, : M * nr * ncl].rearrange(
                    "p (m i j) -> p m i j", m=M, i=nr
                )
                yi = posv[:, :, r0:r1, 1].unsqueeze(3).broadcast_to([P, M, nr, ncl])
                yj = posv[:, :, r0:, 1].unsqueeze(2).broadcast_to([P, M, nr, ncl])
                nc.vector.tensor_sub(dyv, yi, yj)
                actT(nc.scalar.square, reg, reg)
                actT(nc.scalar.square, dyv, dyv)
                nc.vector.tensor_add(reg, reg, dyv)
                # sqrt interleaved per rect: Square and Sqrt share table
                # sets, so the interleave costs no extra table loads.
                actT(nc.scalar.sqrt, reg, reg)
                if idx == 0:
                    # exp + pivot-0 gather/prep for rect 0 immediately, so
                    # pivot 0's reciprocal input is ready long before the
                    # build's DVE work finishes (costs 2 extra table-set
                    # loads per chunk on the idle ACT engine).
                    actT(nc.scalar.activation, reg, reg, AF.Exp, scale=-PHI)
                    alloc_piv(st, 0)
                    prep_piv(st, 0)

        build(0)
        for c in range(chunks):
            st = states.pop(c)
            A4 = st["A4"]
            piv = st["piv"]
            M, off = st["M"], st["off"]

            def rect_reg(r0, r1):
                return A4[:, :, r0:r1, r0:]

            # ---- sweep all 20 pivots ----
            for k in range(N):
                last = k == N - 1
                if last and c + 1 < chunks:
                    # emit the next chunk's build here: its DVE work fills
                    # the gap while ACT runs this chunk's tail + its own
                    # sqrt/exp, and pivot 0 of the next chunk starts with
                    # everything prepped.
                    build(c + 1)
                c3, cr3, rK, rT = piv.pop(k)

                # r = 1/(A[k,k] + TAU)  (nugget applied at pivot time)
                nc.vector.reciprocal(rK, rT)
                # diag <- -r (the final pivot's reversed update negates it)
                act(nc.scalar.mul, A4[:, :, k, k], rK, -1.0)

                if k == 0:
                    # reversed: smallest rects' exps first, matching pivot
                    # 0's rect execution order so ACT stays ahead of DVE
                    for (r0_, r1_) in rects[:0:-1]:
                        actT(nc.scalar.activation,
                            rect_reg(r0_, r1_), rect_reg(r0_, r1_), AF.Exp,
                            scale=-PHI)

                rb = rK.unsqueeze(2).broadcast_to([P, M, N])
                nc.vector.tensor_mul(cr3, c3, rb)

                if last:
                    # pivot col <- cr before the reversed update negates it
                    act(nc.scalar.copy, A4[:, :, :k, k], cr3[:, :, :k])

                # rank-1 update of the upper triangle (rect cover); at the
                # final pivot the subtraction is reversed, which emits the
                # negated matrix (= the inverse) directly.  The rect holding
                # row k runs first (then the one holding row k+1) so the
                # pivot-row write and the next pivot's gather/prep (ACT)
                # complete while the remaining rects are still running.
                first = list(dict.fromkeys(
                    [k // 2, min((k + 1) // 2, len(rects) - 1)]
                ))
                if k == 0:
                    # descending sizes after rect 0: the exp for each rect
                    # (produced smallest-first on ACT) is ready in time
                    order = first + list(range(len(rects) - 1, 0, -1))
                else:
                    order = first + [
                        j for j in range(len(rects)) if j not in first
                    ]
                for ri in order:
                    r0, r1 = rects[ri]
                    nr, ncl = r1 - r0, N - r0
                    tmp = rect_pool.tile([P, MX * RECT_H * N], F32, tag="rect")
                    tv = tmp[:, : M * nr * ncl].rearrange(
                        "p (m i j) -> p m i j", m=M, i=nr
                    )
                    cb = c3[:, :, r0:r1].unsqueeze(3).broadcast_to([P, M, nr, ncl])
                    crb = cr3[:, :, r0:].unsqueeze(2).broadcast_to([P, M, nr, ncl])
                    reg = rect_reg(r0, r1)
                    nc.vector.tensor_mul(tv, cb, crb)
                    if last:
                        nc.vector.tensor_sub(reg, tv, reg)
                        # mirror the rows this rect finalized (ACT)
                        for i in (2 * ri, 2 * ri + 1):
                            if i < N - 1:
                                act(nc.scalar.copy,
                                    A4[:, :, i + 1 :, i], A4[:, :, i, i + 1 :])
                    else:
                        nc.vector.tensor_sub(reg, reg, tv)
                        if ri == k // 2:
                            # pivot row <- cr.  The rank-1 update left it
                            # unchanged (c[k] = cr[k] = 0), so writing after
                            # the rect is equivalent and runs on ACT.
                            act(nc.scalar.copy,
                                A4[:, :, k, k + 1 :], cr3[:, :, k + 1 :])
                        if ri == (k + 1) // 2:
                            alloc_piv(st, k + 1)
                            prep_piv(st, k + 1)

                if not last:
                    # column-part gather for pivot k+1, split along the rect
                    # cover: each piece depends on a single rect update above
                    # and completes while the later rects are still running.
                    cn3 = piv[k + 1][0]
                    for (r0, r1) in rects:
                        lo, hi = r0, min(r1, k + 1)
                        if lo >= hi:
                            break
                        act(nc.scalar.copy, cn3[:, :, lo:hi],
                            A4[:, :, lo:hi, k + 1])
                    if k:
                        # pivot col <- cr (ACT, after all rects that touch it)
                        act(nc.scalar.copy, A4[:, :, :k, k], cr3[:, :, :k])

            # output DMA split by row-halves: the top half's rows (updates +
            # mirrors) finalize partway through pivot 19, so its transfer
            # overlaps the remaining updates (matters for the last chunk's
            # exposed tail).
            H2 = N // 2
            nc.sync.dma_start(out_v[:, off : off + M, :H2, :], A4[:, :, :H2, :])
            nc.sync.dma_start(out_v[:, off : off + M, H2:, :], A4[:, :, H2:, :])


_CACHE = {}


def _m_list_for(b_core, m_chunk):
    T = b_core // P
    if m_chunk is None:
        # 3 chunks: the fewest whose two live A tiles (2 * MX * 1600B) still
        # fit in SBUF alongside the working tiles
        base = [T // 3] * 3
        base[0] += T - 3 * (T // 3)
        return [m for m in base if m]
    return [m_chunk] * (T // m_chunk)


def build_nc(b_core=B_CORE, m_chunk=None, num_devices=N_CORES):
    key = (b_core, m_chunk, num_devices)
    if key in _CACHE:
        return _CACHE[key]
    nc = bacc.Bacc(
        "TRN2", target_bir_lowering=False, debug=False, num_devices=num_devices
    )
    pos_d = nc.dram_tensor("pos", [b_core, N * D], F32, kind="ExternalInput")
    out_d = nc.dram_tensor("out", [b_core, N * N], F32, kind="ExternalOutput")
    with tile.TileContext(nc) as tc:
        emit_kernel(tc, pos_d.ap(), out_d.ap(), b_core, _m_list_for(b_core, m_chunk))
    nc.compile()
    _CACHE[key] = nc
    return nc


def run(pos_full, b_core=B_CORE, m_chunk=None, n_cores=N_CORES, **kw):
    """pos_full: [n_cores*b_core, 20, 2] f32 -> [n_cores*b_core, 20, 20] f32."""
    nc = build_nc(b_core, m_chunk, n_cores)
    flat = np.ascontiguousarray(
        np.asarray(pos_full, dtype=np.float32).reshape(-1, N * D)
    )
    in_maps = [
        {"pos": flat[i * b_core : (i + 1) * b_core]} for i in range(n_cores)
    ]
    res = run_bass_kernel_spmd(nc, in_maps, core_ids=list(range(n_cores)), **kw)
    out = np.concatenate([r["out"] for r in res.results], axis=0)
    return out.reshape(-1, N, N), res


def kernel(neighbor_positions, edge_list=None):
    out, _ = run(neighbor_positions)
    return out


# revision 23
# speedup vs baseline: 1.0138x; 1.0007x over previous
"""Batched 20x20 SPD covariance-matrix inversion on 8 Trainium2 NeuronCores.

For each of 131072 batches: build C = exp(-1.5 * pairwise_dist(pos)) + 0.01*I
from 20 2-D points, return C^{-1}.

Strategy (per core, data-parallel over batch):
 - batch-major layout: each of 128 SBUF partitions holds M matrices' full
   20x20 (400 fp32) in the free dim; 3 chunks (M = 44/42/42) per partition
   (fewer chunks amortize the fixed ~58-cycle DVE per-op init over more
   matrices; 3 is the most that fits two M-sized A tiles in SBUF).
 - symmetric sweep operator (Gauss-Jordan preserving symmetry): only the
   upper triangle is updated each pivot, covered by 2-row rectangles
   (minimizes wasted elements vs per-op overhead on the DVE).
 - the covariance is built on the upper triangle only; the nugget TAU is
   not stored in the matrix but added when the pivot reciprocal is taken
   (exact: the diagonal offset rides additively through the sweep).
 - the final negate is folded into pivot 19 by reversing the subtraction;
   the lower-triangle mirror runs on the ACT engine interleaved with
   pivot 19's updates, and the output DMA is split into row-halves so the
   first half streams out while the second is still being updated.
 - DVE does only the reciprocal, c*r scaling, and the rank-1 updates; all
   gathers (split per-rect so they complete during the previous pivot's
   updates), the nugget add, pivot-slot zeroing, diagonal/pivot-row/col
   writes, and the mirror run on the otherwise-idle ACT engine.
 - table-sensitive ACT ops (Square/Sqrt/Exp) are chained in emission order
   so the scheduler cannot interleave Sqrt and Exp (a table-set switch
   costs ~2.7us); Copy/Identity ops live in every set and schedule freely.
 - each chunk's build is emitted during the previous chunk's final pivot
   so both engines stay busy across chunk transitions.
"""

import numpy as np

import concourse.bass as bass  # noqa: F401  (registers engine APIs)
import concourse.tile as tile
from concourse import bacc, mybir
from concourse.bass_utils import run_bass_kernel_spmd

N = 20                  # matrix dim
D = 2                   # coord dim
PHI = 1.5
TAU = 0.01
P = 128                 # SBUF partitions
N_CORES = 8
B_TOTAL = 131072
B_CORE = B_TOTAL // N_CORES   # 16384

F32 = mybir.dt.float32
AF = mybir.ActivationFunctionType
OP = mybir.AluOpType

RECT_H = 2              # rect-cover row height (2 minimizes DVE cycles)


def emit_kernel(tc, pos_ap, out_ap, b_core, m_list):
    """Emit the per-core program. pos: [b_core, 40] f32, out: [b_core, 400]
    f32.  m_list: matrices per partition per chunk (sum = b_core // 128)."""
    nc = tc.nc
    T = b_core // P
    assert sum(m_list) == T and b_core == P * T
    MX = max(m_list)
    chunks = len(m_list)
    offs = [sum(m_list[:i]) for i in range(chunks)]
    rects = [(a, min(a + RECT_H, N)) for a in range(0, N, RECT_H)]

    def act(fn, *args, **kw):
        """ACT op whose function (Copy/Identity) is present in every
        activation table set — safe to schedule freely."""
        return fn(*args, **kw)

    def actT(fn, *args, **kw):
        """Table-sensitive ACT op (Square/Sqrt/Exp), chained in emission
        order so the scheduler cannot interleave Sqrt and Exp (each
        table-set switch costs ~2.7us)."""
        inst = fn(*args, **kw)
        tc.chain_iter_dep("actq_t", inst.ins)
        return inst

    pos_v = pos_ap.rearrange("(p t) f -> p (t f)", p=P)
    out_v = out_ap.rearrange("(p t) (i j) -> p t i j", p=P, i=N)

    with (
        tc.tile_pool(name="const", bufs=1) as const_pool,
        tc.tile_pool(name="pos", bufs=2) as pos_pool,
        tc.tile_pool(name="A", bufs=2) as a_pool,
        tc.tile_pool(name="dy", bufs=2) as dy_pool,
        tc.tile_pool(name="rect", bufs=2) as rect_pool,
        tc.tile_pool(name="small", bufs=2) as small_pool,
    ):
        tau_t = const_pool.tile([P, 1], F32)
        nc.vector.memset(tau_t[:, :], TAU)

        states = {}

        def alloc_piv(st, k):
            M = st["M"]
            cK = small_pool.tile([P, MX * N], F32, tag="c")
            crK = small_pool.tile([P, MX * N], F32, tag="cr")
            rK = small_pool.tile([P, MX], F32, tag="r")
            rT = small_pool.tile([P, MX], F32, tag="rt")
            st["piv"][k] = (
                cK[:, : M * N].rearrange("p (m i) -> p m i", m=M),
                crK[:, : M * N].rearrange("p (m i) -> p m i", m=M),
                rK[:, :M], rT[:, :M],
            )

        def prep_piv(st, k):
            """Row-part gather + nugget add + pivot-slot zero for pivot k
            (ACT).  Emitted as soon as row k's update has been emitted so
            the DVE reciprocal never waits."""
            A4 = st["A4"]
            c3, _, _, rT = st["piv"][k]
            act(nc.scalar.copy, c3[:, :, k:], A4[:, :, k, k:])
            act(nc.scalar.activation, rT, c3[:, :, k], AF.Identity, tau_t[:, :])
            act(nc.scalar.mul, c3[:, :, k], c3[:, :, k], 0.0)  # zero slot

        def build(c):
            """Covariance build for chunk c (upper rects only):
            A = d^2 -> d -> exp, plus pivot-0 gather/prep.  Called one
            pivot early (during chunk c-1's last pivot) so the chunk
            transition keeps both engines busy."""
            M, off = m_list[c], offs[c]
            pos_t = pos_pool.tile([P, MX * N * D], F32)
            nc.sync.dma_start(
                pos_t[:, : M * N * D], pos_v[:, off * N * D : (off + M) * N * D]
            )
            posv = pos_t[:, : M * N * D].rearrange("p (m i d) -> p m i d", m=M, i=N)
            A = a_pool.tile([P, MX * N * N], F32)
            A4 = A[:, : M * N * N].rearrange("p (m i j) -> p m i j", m=M, i=N)
            st = {"A4": A4, "piv": {}, "M": M, "off": off}
            states[c] = st

            for idx, (r0, r1) in enumerate(rects):
                nr, ncl = r1 - r0, N - r0
                reg = A4[:, :, r0:r1, r0:]
                xi = posv[:, :, r0:r1, 0].unsqueeze(3).broadcast_to([P, M, nr, ncl])
                xj = posv[:, :, r0:, 0].unsqueeze(2).broadcast_to([P, M, nr, ncl])
                nc.vector.tensor_sub(reg, xi, xj)
                dy = dy_pool.tile([P, MX * RECT_H * N], F32, tag="dy")
                dyv = dy[:, : M * nr * ncl].rearrange(
                    "p (m i j) -> p m i j", m=M, i=nr
                )
                yi = posv[:, :, r0:r1, 1].unsqueeze(3).broadcast_to([P, M, nr, ncl])
                yj = posv[:, :, r0:, 1].unsqueeze(2).broadcast_to([P, M, nr, ncl])
                nc.vector.tensor_sub(dyv, yi, yj)
                actT(nc.scalar.square, reg, reg)
                actT(nc.scalar.square, dyv, dyv)
                nc.vector.tensor_add(reg, reg, dyv)
                # sqrt interleaved per rect: Square and Sqrt share table
                # sets, so the interleave costs no extra table loads.
                actT(nc.scalar.sqrt, reg, reg)
                if idx == 0:
                    # exp + pivot-0 gather/prep for rect 0 immediately, so
                    # pivot 0's reciprocal input is ready long before the
                    # build's DVE work finishes (costs 2 extra table-set
                    # loads per chunk on the idle ACT engine).
                    actT(nc.scalar.activation, reg, reg, AF.Exp, scale=-PHI)
                    alloc_piv(st, 0)
                    prep_piv(st, 0)

        build(0)
        for c in range(chunks):
            st = states.pop(c)
            A4 = st["A4"]
            piv = st["piv"]
            M, off = st["M"], st["off"]

            def rect_reg(r0, r1):
                return A4[:, :, r0:r1, r0:]

            # ---- sweep all 20 pivots ----
            for k in range(N):
                last = k == N - 1
                if last and c + 1 < chunks:
                    # emit the next chunk's build here: its DVE work fills
                    # the gap while ACT runs this chunk's tail + its own
                    # sqrt/exp, and pivot 0 of the next chunk starts with
                    # everything prepped.
                    build(c + 1)
                c3, cr3, rK, rT = piv.pop(k)

                # r = 1/(A[k,k] + TAU)  (nugget applied at pivot time)
                nc.vector.reciprocal(rK, rT)
                # diag <- -r (the final pivot's reversed update negates it)
                act(nc.scalar.mul, A4[:, :, k, k], rK, -1.0)

                if k == 0:
                    # reversed: smallest rects' exps first, matching pivot
                    # 0's rect execution order so ACT stays ahead of DVE
                    for (r0_, r1_) in rects[:0:-1]:
                        actT(nc.scalar.activation,
                            rect_reg(r0_, r1_), rect_reg(r0_, r1_), AF.Exp,
                            scale=-PHI)

                rb = rK.unsqueeze(2).broadcast_to([P, M, N])
                nc.vector.tensor_mul(cr3, c3, rb)

                if last:
                    # pivot col <- cr before the reversed update negates it
                    act(nc.scalar.copy, A4[:, :, :k, k], cr3[:, :, :k])

                # rank-1 update of the upper triangle (rect cover); at the
                # final pivot the subtraction is reversed, which emits the
                # negated matrix (= the inverse) directly.  The rect holding
                # row k runs first (then the one holding row k+1) so the
                # pivot-row write and the next pivot's gather/prep (ACT)
                # complete while the remaining rects are still running.
                first = list(dict.fromkeys(
                    [k // 2, min((k + 1) // 2, len(rects) - 1)]
                ))
                if k == 0:
                    # descending sizes after rect 0: the exp for each rect
                    # (produced smallest-first on ACT) is ready in time
                    order = first + list(range(len(rects) - 1, 0, -1))
                else:
                    order = first + [
                        j for j in range(len(rects)) if j not in first
                    ]
                for ri in order:
                    r0, r1 = rects[ri]
                    nr, ncl = r1 - r0, N - r0
                    tmp = rect_pool.tile([P, MX * RECT_H * N], F32, tag="rect")
                    tv = tmp[:, : M * nr * ncl].rearrange(
                        "p (m i j) -> p m i j", m=M, i=nr
                    )
                    cb = c3[:, :, r0:r1].unsqueeze(3).broadcast_to([P, M, nr, ncl])
                    crb = cr3[:, :, r0:].unsqueeze(2).broadcast_to([P, M, nr, ncl])
                    reg = rect_reg(r0, r1)
                    nc.vector.tensor_mul(tv, cb, crb)
                    if last:
                        nc.vector.tensor_sub(reg, tv, reg)
                        # mirror the rows this rect finalized (ACT)
                        for i in (2 * ri, 2 * ri + 1):
                            if i < N - 1:
                                act(nc.scalar.copy,
                                    A4[:, :, i + 1 :, i], A4[:, :, i, i + 1 :])
                    else:
                        nc.vector.tensor_sub(reg, reg, tv)
                        if ri == k // 2:
                            # pivot row <- cr.  The rank-1 update left it
                            # unchanged (c[k] = cr[k] = 0), so writing after
                            # the rect is equivalent and runs on ACT.
                            act(nc.scalar.copy,
                                A4[:, :, k, k + 1 :], cr3[:, :, k + 1 :])
                        if ri == (k + 1) // 2:
                            alloc_piv(st, k + 1)
                            prep_piv(st, k + 1)

                if not last:
                    # column-part gather for pivot k+1, split along the rect
                    # cover: each piece depends on a single rect update above
                    # and completes while the later rects are still running.
                    cn3 = piv[k + 1][0]
                    for (r0, r1) in rects:
                        lo, hi = r0, min(r1, k + 1)
                        if lo >= hi:
                            break
                        act(nc.scalar.copy, cn3[:, :, lo:hi],
                            A4[:, :, lo:hi, k + 1])
                    if k:
                        # pivot col <- cr (ACT, after all rects that touch it)
                        act(nc.scalar.copy, A4[:, :, :k, k], cr3[:, :, :k])

            # output DMA split by row-halves: the top half's rows (updates +
            # mirrors) finalize partway through pivot 19, so its transfer
            # overlaps the remaining updates (matters for the last chunk's
            # exposed tail).
            H2 = N // 2
            nc.sync.dma_start(out_v[:, off : off + M, :H2, :], A4[:, :, :H2, :])
            nc.sync.dma_start(out_v[:, off : off + M, H2:, :], A4[:, :, H2:, :])


_CACHE = {}


def _m_list_for(b_core, m_chunk):
    T = b_core // P
    if m_chunk is None:
        # 3 chunks: the fewest whose two live A tiles (2 * MX * 1600B) still
        # fit in SBUF alongside the working tiles.  The last chunk is the
        # smallest so the final (exposed) pivot+DMA tail is shortest.
        if T % 16 == 0:
            m = T * 11 // 32
            return [m, m, T - 2 * m]
        lo = T // 3
        return [T - 2 * lo, lo, lo]
    return [m_chunk] * (T // m_chunk)


def build_nc(b_core=B_CORE, m_chunk=None, num_devices=N_CORES):
    key = (b_core, m_chunk, num_devices)
    if key in _CACHE:
        return _CACHE[key]
    nc = bacc.Bacc(
        "TRN2", target_bir_lowering=False, debug=False, num_devices=num_devices
    )
    pos_d = nc.dram_tensor("pos", [b_core, N * D], F32, kind="ExternalInput")
    out_d = nc.dram_tensor("out", [b_core, N * N], F32, kind="ExternalOutput")
    with tile.TileContext(nc) as tc:
        emit_kernel(tc, pos_d.ap(), out_d.ap(), b_core, _m_list_for(b_core, m_chunk))
    nc.compile()
    _CACHE[key] = nc
    return nc


def run(pos_full, b_core=B_CORE, m_chunk=None, n_cores=N_CORES, **kw):
    """pos_full: [n_cores*b_core, 20, 2] f32 -> [n_cores*b_core, 20, 20] f32."""
    nc = build_nc(b_core, m_chunk, n_cores)
    flat = np.ascontiguousarray(
        np.asarray(pos_full, dtype=np.float32).reshape(-1, N * D)
    )
    in_maps = [
        {"pos": flat[i * b_core : (i + 1) * b_core]} for i in range(n_cores)
    ]
    res = run_bass_kernel_spmd(nc, in_maps, core_ids=list(range(n_cores)), **kw)
    out = np.concatenate([r["out"] for r in res.results], axis=0)
    return out.reshape(-1, N, N), res


def kernel(neighbor_positions, edge_list=None):
    out, _ = run(neighbor_positions)
    return out


# revision 37
# speedup vs baseline: 1.0575x; 1.0431x over previous
"""Batched 20x20 SPD covariance-matrix inversion on 8 Trainium2 NeuronCores.

For each of 131072 batches: build C = exp(-1.5 * pairwise_dist(pos)) + 0.01*I
from 20 2-D points, return C^{-1}.

Strategy (per core, data-parallel over batch):
 - batch-major layout: each of 128 SBUF partitions holds M matrices' full
   20x20 (400 fp32) in the free dim; 3 chunks (M = 44/44/40) per partition
   (fewer chunks amortize the fixed ~58-cycle DVE per-op init over more
   matrices; 3 is the most that fits two M-sized A tiles in SBUF).
 - symmetric sweep operator (Gauss-Jordan preserving symmetry): only the
   upper triangle is updated each pivot, covered by 2-row rectangles
   (minimizes wasted elements vs per-op overhead on the DVE).
 - the covariance is built on the upper triangle only; the nugget TAU is
   not stored in the matrix but added when the pivot reciprocal is taken
   (exact: the diagonal offset rides additively through the sweep).
 - the final negate is folded into pivot 19 by reversing the subtraction;
   the lower-triangle mirror runs on the ACT engine interleaved with
   pivot 19's updates, and the output DMA is split into row-halves so the
   first half streams out while the second is still being updated.
 - DVE does only the reciprocal, c*r scaling, and the rank-1 updates; all
   gathers (split per-rect so they complete during the previous pivot's
   updates), the nugget add, pivot-slot zeroing, diagonal/pivot-row/col
   writes, and the mirror run on the otherwise-idle ACT engine.
 - table-sensitive ACT ops (Square/Sqrt/Exp) are chained in emission order
   so the scheduler cannot interleave Sqrt and Exp (a table-set switch
   costs ~2.7us); Copy/Identity ops live in every set and schedule freely.
 - each chunk's build is emitted during the previous chunk's final pivot
   so both engines stay busy across chunk transitions.
"""

import numpy as np

import concourse.bass as bass  # noqa: F401  (registers engine APIs)
import concourse.tile as tile
from concourse import bacc, mybir
from concourse.bass_utils import run_bass_kernel_spmd

N = 20                  # matrix dim
D = 2                   # coord dim
PHI = 1.5
TAU = 0.01
P = 128                 # SBUF partitions
N_CORES = 8
B_TOTAL = 131072
B_CORE = B_TOTAL // N_CORES   # 16384

F32 = mybir.dt.float32
AF = mybir.ActivationFunctionType
OP = mybir.AluOpType

RECT_H = 2              # rect-cover row height (2 minimizes DVE cycles)


def emit_kernel(tc, pos_ap, out_ap, b_core, m_list):
    """Emit the per-core program. pos: [b_core, 40] f32, out: [b_core, 400]
    f32.  m_list: matrices per partition per chunk (sum = b_core // 128)."""
    nc = tc.nc
    T = b_core // P
    assert sum(m_list) == T and b_core == P * T
    MX = max(m_list)
    chunks = len(m_list)
    offs = [sum(m_list[:i]) for i in range(chunks)]
    rects = [(a, min(a + RECT_H, N)) for a in range(0, N, RECT_H)]

    def act(fn, *args, **kw):
        """ACT op whose function (Copy/Identity) is present in every
        activation table set — safe to schedule freely."""
        return fn(*args, **kw)

    def actT(fn, *args, **kw):
        """Table-sensitive ACT op (Square/Sqrt/Exp), chained in emission
        order so the scheduler cannot interleave Sqrt and Exp (each
        table-set switch costs ~2.7us)."""
        inst = fn(*args, **kw)
        tc.chain_iter_dep("actq_t", inst.ins)
        return inst

    pos_v = pos_ap.rearrange("(p t) f -> p (t f)", p=P)
    out_v = out_ap.rearrange("(p t) (i j) -> p t i j", p=P, i=N)

    with (
        tc.tile_pool(name="const", bufs=1) as const_pool,
        tc.tile_pool(name="pos", bufs=2) as pos_pool,
        tc.tile_pool(name="A", bufs=2) as a_pool,
        tc.tile_pool(name="dy", bufs=3) as dy_pool,
        tc.tile_pool(name="rect", bufs=2) as rect_pool,
        tc.tile_pool(name="small", bufs=2) as small_pool,
    ):
        tau_t = const_pool.tile([P, 1], F32)
        nc.vector.memset(tau_t[:, :], TAU)

        states = {}

        def alloc_piv(st, k):
            M = st["M"]
            cK = small_pool.tile([P, MX * N], F32, tag="c")
            crK = small_pool.tile([P, MX * N], F32, tag="cr")
            rK = small_pool.tile([P, MX], F32, tag="r")
            rT = small_pool.tile([P, MX], F32, tag="rt")
            st["piv"][k] = (
                cK[:, : M * N].rearrange("p (m i) -> p m i", m=M),
                crK[:, : M * N].rearrange("p (m i) -> p m i", m=M),
                rK[:, :M], rT[:, :M],
            )

        def prep_piv(st, k):
            """Row-part gather + nugget add + pivot-slot zero for pivot k
            (ACT).  Emitted as soon as row k's update has been emitted so
            the DVE reciprocal never waits."""
            A4 = st["A4"]
            c3, _, _, rT = st["piv"][k]
            act(nc.scalar.copy, c3[:, :, k:], A4[:, :, k, k:])
            act(nc.scalar.activation, rT, c3[:, :, k], AF.Identity, tau_t[:, :])
            act(nc.scalar.mul, c3[:, :, k], c3[:, :, k], 0.0)  # zero slot

        def build(c):
            """Covariance build for chunk c (upper rects only):
            A = d^2 -> d -> exp, plus pivot-0 gather/prep.  Called one
            pivot early (during chunk c-1's last pivot) so the chunk
            transition keeps both engines busy."""
            M, off = m_list[c], offs[c]
            pos_t = pos_pool.tile([P, MX * N * D], F32)
            nc.sync.dma_start(
                pos_t[:, : M * N * D], pos_v[:, off * N * D : (off + M) * N * D]
            )
            posv = pos_t[:, : M * N * D].rearrange("p (m i d) -> p m i d", m=M, i=N)
            A = a_pool.tile([P, MX * N * N], F32)
            A4 = A[:, : M * N * N].rearrange("p (m i j) -> p m i j", m=M, i=N)
            Av = A[:, : M * N * N].rearrange("p (m x) -> p m x", m=M)
            # even-diagonal elements (2r, 2r), excluded from the rect cover
            diagE = Av[:, :, 0 : N * N : 2 * (N + 1)]
            st = {"A4": A4, "diagE": diagE, "piv": {}, "M": M, "off": off}
            states[c] = st

            # even diagonal of the covariance is exp(0) = 1 (the rects only
            # cover columns > r0; odd diagonals are inside their rect)
            act(nc.scalar.activation, diagE, diagE, AF.Identity,
                bias=1.0, scale=0.0)

            for idx, (r0, r1) in enumerate(rects):
                nr, ncl = r1 - r0, N - r0 - 1
                reg = A4[:, :, r0:r1, r0 + 1 :]
                xi = posv[:, :, r0:r1, 0].unsqueeze(3).broadcast_to([P, M, nr, ncl])
                xj = posv[:, :, r0 + 1 :, 0].unsqueeze(2).broadcast_to(
                    [P, M, nr, ncl]
                )
                nc.vector.tensor_sub(reg, xi, xj)
                dy = dy_pool.tile([P, MX * RECT_H * N], F32, tag="dy")
                dyv = dy[:, : M * nr * ncl].rearrange(
                    "p (m i j) -> p m i j", m=M, i=nr
                )
                yi = posv[:, :, r0:r1, 1].unsqueeze(3).broadcast_to([P, M, nr, ncl])
                yj = posv[:, :, r0 + 1 :, 1].unsqueeze(2).broadcast_to(
                    [P, M, nr, ncl]
                )
                nc.vector.tensor_sub(dyv, yi, yj)
                actT(nc.scalar.square, reg, reg)
                actT(nc.scalar.square, dyv, dyv)
                nc.vector.tensor_add(reg, reg, dyv)
                # sqrt interleaved per rect: Square and Sqrt share table
                # sets, so the interleave costs no extra table loads.
                actT(nc.scalar.sqrt, reg, reg)
                if idx == 0:
                    # exp + pivot-0 gather/prep for rect 0 immediately, so
                    # pivot 0's reciprocal input is ready long before the
                    # build's DVE work finishes (costs 2 extra table-set
                    # loads per chunk on the idle ACT engine).
                    actT(nc.scalar.activation, reg, reg, AF.Exp, scale=-PHI)
                    alloc_piv(st, 0)
                    prep_piv(st, 0)

        build(0)
        for c in range(chunks):
            st = states.pop(c)
            A4 = st["A4"]
            diagE = st["diagE"]
            piv = st["piv"]
            M, off = st["M"], st["off"]

            def rect_reg(r0, r1):
                return A4[:, :, r0:r1, r0 + 1 :]

            # ---- sweep all 20 pivots ----
            for k in range(N):
                last = k == N - 1
                if last and c + 1 < chunks:
                    # emit the next chunk's build here: its DVE work fills
                    # the gap while ACT runs this chunk's tail + its own
                    # sqrt/exp, and pivot 0 of the next chunk starts with
                    # everything prepped.
                    build(c + 1)
                c3, cr3, rK, rT = piv.pop(k)

                # r = 1/(A[k,k] + TAU)  (nugget applied at pivot time)
                nc.vector.reciprocal(rK, rT)
                # diag <- -r (the final pivot's reversed update negates it)
                act(nc.scalar.mul, A4[:, :, k, k], rK, -1.0)

                if k == 0:
                    # reversed: smallest rects' exps first, matching pivot
                    # 0's rect execution order so ACT stays ahead of DVE
                    for (r0_, r1_) in rects[:0:-1]:
                        actT(nc.scalar.activation,
                            rect_reg(r0_, r1_), rect_reg(r0_, r1_), AF.Exp,
                            scale=-PHI)

                rb = rK.unsqueeze(2).broadcast_to([P, M, N])
                nc.vector.tensor_mul(cr3, c3, rb)

                # rank-1 update of the even-diagonal elements (2r, 2r) —
                # the rect cover below excludes them (it would waste one
                # below-diagonal element per rect otherwise).  One strided
                # mul/sub pair covers all ten.
                dt = small_pool.tile([P, MX * (N // 2)], F32, tag="dg")
                dv = dt[:, : M * (N // 2)].rearrange("p (m r) -> p m r", m=M)
                nc.vector.tensor_mul(dv, c3[:, :, 0:N:2], cr3[:, :, 0:N:2])
                if last:
                    nc.vector.tensor_sub(diagE, dv, diagE)
                    # pivot col <- cr before the reversed update negates it
                    act(nc.scalar.copy, A4[:, :, :k, k], cr3[:, :, :k])
                else:
                    nc.vector.tensor_sub(diagE, diagE, dv)

                # rank-1 update of the upper triangle (rect cover); at the
                # final pivot the subtraction is reversed, which emits the
                # negated matrix (= the inverse) directly.  The rect holding
                # row k runs first (then the one holding row k+1) so the
                # pivot-row write and the next pivot's gather/prep (ACT)
                # complete while the remaining rects are still running.
                first = list(dict.fromkeys(
                    [k // 2, min((k + 1) // 2, len(rects) - 1)]
                ))
                if k == 0:
                    # descending sizes after rect 0: the exp for each rect
                    # (produced smallest-first on ACT) is ready in time
                    order = first + list(range(len(rects) - 1, 0, -1))
                else:
                    order = first + [
                        j for j in range(len(rects)) if j not in first
                    ]
                for ri in order:
                    r0, r1 = rects[ri]
                    nr, ncl = r1 - r0, N - r0 - 1
                    tmp = rect_pool.tile([P, MX * RECT_H * N], F32, tag="rect")
                    tv = tmp[:, : M * nr * ncl].rearrange(
                        "p (m i j) -> p m i j", m=M, i=nr
                    )
                    cb = c3[:, :, r0:r1].unsqueeze(3).broadcast_to([P, M, nr, ncl])
                    crb = cr3[:, :, r0 + 1 :].unsqueeze(2).broadcast_to(
                        [P, M, nr, ncl]
                    )
                    reg = rect_reg(r0, r1)
                    nc.vector.tensor_mul(tv, cb, crb)
                    if last:
                        nc.vector.tensor_sub(reg, tv, reg)
                        # mirror the rows this rect finalized (ACT)
                        for i in (2 * ri, 2 * ri + 1):
                            if i < N - 1:
                                act(nc.scalar.copy,
                                    A4[:, :, i + 1 :, i], A4[:, :, i, i + 1 :])
                    else:
                        nc.vector.tensor_sub(reg, reg, tv)
                        if ri == k // 2:
                            # pivot row <- cr.  The rank-1 update left it
                            # unchanged (c[k] = cr[k] = 0), so writing after
                            # the rect is equivalent and runs on ACT.
                            act(nc.scalar.copy,
                                A4[:, :, k, k + 1 :], cr3[:, :, k + 1 :])
                        if ri == (k + 1) // 2:
                            alloc_piv(st, k + 1)
                            prep_piv(st, k + 1)

                if not last:
                    # column-part gather for pivot k+1, split along the rect
                    # cover: each piece depends on a single rect update above
                    # and completes while the later rects are still running.
                    cn3 = piv[k + 1][0]
                    for (r0, r1) in rects:
                        lo, hi = r0, min(r1, k + 1)
                        if lo >= hi:
                            break
                        act(nc.scalar.copy, cn3[:, :, lo:hi],
                            A4[:, :, lo:hi, k + 1])
                    if k:
                        # pivot col <- cr (ACT, after all rects that touch it)
                        act(nc.scalar.copy, A4[:, :, :k, k], cr3[:, :, :k])

            # output DMA split by row-halves: the top half's rows (updates +
            # mirrors) finalize partway through pivot 19, so its transfer
            # overlaps the remaining updates (matters for the last chunk's
            # exposed tail).
            H2 = N // 2
            nc.sync.dma_start(out_v[:, off : off + M, :H2, :], A4[:, :, :H2, :])
            nc.sync.dma_start(out_v[:, off : off + M, H2:, :], A4[:, :, H2:, :])


_CACHE = {}


def _m_list_for(b_core, m_chunk):
    T = b_core // P
    if m_chunk is None:
        # 3 chunks: the fewest whose two live A tiles (2 * MX * 1600B) still
        # fit in SBUF alongside the working tiles.  The last chunk is the
        # smallest so the final (exposed) pivot+DMA tail is shortest.
        if T % 16 == 0:
            m = T * 11 // 32
            return [m, m, T - 2 * m]
        lo = T // 3
        return [T - 2 * lo, lo, lo]
    return [m_chunk] * (T // m_chunk)


def build_nc(b_core=B_CORE, m_chunk=None, num_devices=N_CORES):
    key = (b_core, m_chunk, num_devices)
    if key in _CACHE:
        return _CACHE[key]
    nc = bacc.Bacc(
        "TRN2", target_bir_lowering=False, debug=False, num_devices=num_devices
    )
    pos_d = nc.dram_tensor("pos", [b_core, N * D], F32, kind="ExternalInput")
    out_d = nc.dram_tensor("out", [b_core, N * N], F32, kind="ExternalOutput")
    with tile.TileContext(nc) as tc:
        emit_kernel(tc, pos_d.ap(), out_d.ap(), b_core, _m_list_for(b_core, m_chunk))
    nc.compile()
    _CACHE[key] = nc
    return nc


def run(pos_full, b_core=B_CORE, m_chunk=None, n_cores=N_CORES, **kw):
    """pos_full: [n_cores*b_core, 20, 2] f32 -> [n_cores*b_core, 20, 20] f32."""
    nc = build_nc(b_core, m_chunk, n_cores)
    flat = np.ascontiguousarray(
        np.asarray(pos_full, dtype=np.float32).reshape(-1, N * D)
    )
    in_maps = [
        {"pos": flat[i * b_core : (i + 1) * b_core]} for i in range(n_cores)
    ]
    res = run_bass_kernel_spmd(nc, in_maps, core_ids=list(range(n_cores)), **kw)
    out = np.concatenate([r["out"] for r in res.results], axis=0)
    return out.reshape(-1, N, N), res


def kernel(neighbor_positions, edge_list=None):
    out, _ = run(neighbor_positions)
    return out
